# revision 62
# baseline (speedup 1.0000x reference)
"""Self-contained distributed Bass kernel for nn_Atom_Gloal_37958920962359.

Two-layer GCN (PyG GCNConv semantics) + batchnorm + global max pool over
200k nodes / 800k edges / 8192 graphs, plus a cell-line MLP branch, running
SPMD on 8 TRN2 NeuronCores.

v2 strategy (fp16 + host-built scatter matrices + merged gathers):
- graph-aligned node/edge shards per core; GCN as z = [Dinv (A+I) Dinv] x,
  u = z @ W + b so aggregation commutes with the weight matmul.
- aggregation per 256-dst tile: one fp16 matmul per 128-slot chunk with the
  one-hot scatter matrix S (incl. self-loop chunks) PRECOMPUTED ON HOST and
  streamed from DRAM; gathered source rows fetched with ONE indirect DMA per
  4-tile group (SWDGE fixed cost amortized).
- all matmul operands fp16 (PSUM accumulates fp32); tables (x, u1, u2pool)
  in fp16 so gather/AllGather bytes halve.
- BN affines folded into downstream matmuls (stats via ACT accum_out +
  small AllReduce + pad correction); layer-1 table replicated via fp16
  AllGather; pool via fp16 dma_gather with graphs bucketed by padded node
  count into uniform segmented max-reduces.
"""
import sys
sys.path.insert(0, "/opt/trn_rl_repo")

import numpy as np
from contextlib import ExitStack

import concourse.bass as bass
import concourse.bacc as bacc
import concourse.mybir as mybir
import concourse.tile as tile
from concourse.masks import make_identity
from concourse.bass_utils import run_bass_kernel_spmd


N_NODES = 200000
N_EDGES = 800000
N_GRAPHS = 8192
DIM_DRUG = 128
HID = 128
OUT = 256
DIM_CELL = 954
EPS = 1e-5
N_CORES = 8
TILE_DST = 256      # dst nodes per psum tile
CHUNK = 128         # slots per matmul chunk (K dim)
GPC = N_GRAPHS // N_CORES  # graphs per core
POOL_SUB = 8        # pool group size (level-1 max granularity)
GMAX = 3            # tiles per gather group
NSELF = 2           # self-loop chunks per tile (TILE_DST // CHUNK)


def build_plan(drug_adj, ibatch):
    """All index preprocessing + host-built fp16 scatter matrices."""
    ibatch = np.asarray(ibatch)
    src_all = np.asarray(drug_adj[0]).astype(np.int64)
    dst_all = np.asarray(drug_adj[1]).astype(np.int64)

    node_start = np.searchsorted(ibatch, np.arange(N_CORES + 1) * GPC).astype(np.int64)
    nodes_c = np.diff(node_start)

    deg = np.bincount(dst_all, minlength=N_NODES).astype(np.int64) + 1
    dinv = 1.0 / np.sqrt(deg.astype(np.float64))

    owner_of_node = np.searchsorted(node_start, np.arange(N_NODES), side="right") - 1
    edge_owner = owner_of_node[dst_all]

    NT = int(np.max(np.ceil(nodes_c / TILE_DST)).astype(int))
    SH = NT * TILE_DST
    NGRP = (NT + GMAX - 1) // GMAX

    cores = []
    maxC = 0
    per_core_slots = []
    for c in range(N_CORES):
        lo, hi = node_start[c], node_start[c + 1]
        m = edge_owner == c
        s_c = src_all[m]
        d_c = dst_all[m]
        order = np.argsort(d_c, kind="stable")
        s_c, d_c = s_c[order], d_c[order]
        dloc = d_c - lo
        tile_of_slot = dloc // TILE_DST
        cnt = np.bincount(tile_of_slot, minlength=NT)
        loc_mask = (s_c >= lo) & (s_c < hi)
        nloc0 = np.minimum(np.bincount(tile_of_slot[loc_mask], minlength=NT), CHUNK)
        # chunk 0 of each tile reserved for (up to 128) local-src slots so
        # its L2 gather reads u1_shard and needs no AllGather
        maxC = max(maxC, 1 + int(np.ceil((cnt - nloc0) / CHUNK).max()))
        per_core_slots.append((s_c, dloc, cnt, loc_mask, lo, hi))

    C = maxC
    spt = C * CHUNK  # slots per tile
    KCH = NSELF + C  # chunks per tile incl self

    for c in range(N_CORES):
        s_c, dloc, cnt, loc_mask, lo, hi = per_core_slots[c]
        nreal = int(hi - lo)
        total_slots = NT * spt
        src_slots = np.zeros(total_slots, dtype=np.int64)
        dstl_slots = np.full(total_slots, -1, dtype=np.int64)
        scale_slots = np.zeros(total_slots, dtype=np.float32)
        tile_offsets = np.zeros(NT + 1, dtype=np.int64)
        tile_offsets[1:] = np.cumsum(cnt)
        for t in range(NT):
            a, b = tile_offsets[t], tile_offsets[t + 1]
            base = t * spt
            lm = loc_mask[a:b]
            li = np.nonzero(lm)[0]
            n0 = min(CHUNK, len(li))
            first = li[:n0]
            rest = np.concatenate([li[n0:], np.nonzero(~lm)[0]])
            for pos, sel in ((base, first), (base + CHUNK, rest)):
                k = len(sel)
                src_slots[pos:pos + k] = s_c[a:b][sel]
                dstl_slots[pos:pos + k] = dloc[a:b][sel] - t * TILE_DST
                scale_slots[pos:pos + k] = (
                    dinv[s_c[a:b][sel]] * dinv[dloc[a:b][sel] + lo]).astype(np.float32)

        # d2 (self-loop scale) and s-vector per padded shard row
        d2 = np.zeros(SH, dtype=np.float32)
        d2[:nreal] = (dinv[lo:hi] ** 2).astype(np.float32)
        s_vec = np.zeros(SH, dtype=np.float64)
        vmask = dstl_slots >= 0
        np.add.at(s_vec, (np.arange(total_slots) // spt) * TILE_DST + np.where(vmask, dstl_slots, 0),
                  np.where(vmask, scale_slots.astype(np.float64), 0.0))
        s_vec += d2.astype(np.float64)

        # scatter matrices S [NT, KCH, 128, TD] fp16 (self chunks first)
        S = np.zeros((NT, KCH, CHUNK, TILE_DST), dtype=np.float16)
        idxs = np.nonzero(vmask)[0]
        tt = idxs // spt
        rr = idxs % spt
        jj = rr // CHUNK
        pp = rr % CHUNK
        S[tt, NSELF + jj, pp, dstl_slots[idxs]] = scale_slots[idxs].astype(np.float16)
        rowl = np.arange(SH)
        tts = rowl // TILE_DST
        rrs = rowl % TILE_DST
        S[tts, rrs // CHUNK, rrs % CHUNK, rrs] = d2.astype(np.float16)
        pad = NGRP * GMAX - NT
        if pad:
            S = np.concatenate([S, np.zeros((pad, KCH, CHUNK, TILE_DST), np.float16)], 0)
        Sw = np.ascontiguousarray(
            S.reshape(NGRP, GMAX, KCH, CHUNK, TILE_DST)
            .transpose(0, 3, 1, 2, 4)
            .reshape(NGRP, CHUNK, GMAX * KCH * TILE_DST))

        # gather index pack [128, NT*C] for L2
        def pack(arr):
            return np.ascontiguousarray(
                arr.reshape(NT, C, CHUNK).transpose(2, 0, 1).reshape(CHUNK, NT * C)
            ).astype(np.int32)

        own = owner_of_node[src_slots]
        l2_rows = own * SH + (src_slots - node_start[own])
        idx_l2 = pack(l2_rows)
        # chunk-0 local row indices into u1_shard [128, NT]
        c0 = src_slots.reshape(NT, C, CHUNK)[:, 0, :]  # [NT, 128]
        c0v = dstl_slots.reshape(NT, C, CHUNK)[:, 0, :] >= 0
        idx_loc = np.ascontiguousarray(
            np.where(c0v, c0 - lo, 0).T).astype(np.int32)

        cores.append(dict(
            lo=int(lo), hi=int(hi), nreal=nreal,
            src_slots=src_slots, idx_l2=idx_l2, idx_loc=idx_loc, Sw=Sw,
            s_flat=s_vec.astype(np.float16),
        ))

    # ---- pool layout (same as baseline) ----
    gcnt = np.bincount(ibatch, minlength=N_GRAPHS)
    maxg = int(gcnt.max())
    KMAX = int(np.ceil(maxg / POOL_SUB))
    pool = dict(KMAX=KMAX, maxg=maxg)
    bucket_counts = np.zeros((N_CORES, KMAX + 1), dtype=np.int64)
    for c in range(N_CORES):
        g0 = c * GPC
        kg = np.ceil(gcnt[g0:g0 + GPC] / POOL_SUB).astype(np.int64)
        for kk in range(1, KMAX + 1):
            bucket_counts[c, kk] = int((kg == kk).sum())
    pool["caps"] = bucket_counts.max(axis=0)

    return dict(cores=cores, NT=NT, SH=SH, C=C, KCH=KCH, NGRP=NGRP,
                node_start=node_start, dinv=dinv, pool=pool,
                n_pad_nodes=int(N_CORES * SH - N_NODES))


def make_pool_ops(caps, KMAX, max_op_idx=1024):
    from math import gcd
    pool_ops = []
    caps_pad = np.zeros(KMAX + 1, dtype=np.int64)
    col_off = 0
    out_off = 0
    for kk in range(1, KMAX + 1):
        if caps[kk] == 0:
            continue
        L = POOL_SUB * kk
        m = 128 // gcd(L, 128)
        cap = int(np.ceil(caps[kk] / m) * m)
        caps_pad[kk] = cap
        gpo = max(m, (max_op_idx // L) // m * m)
        g = 0
        while g < cap:
            ng = int(min(gpo, cap - g))
            n_idx = ng * L
            assert n_idx % 128 == 0
            pool_ops.append((n_idx, ng, L, col_off, out_off))
            col_off += n_idx // 16
            out_off += ng
            g += ng
    return pool_ops, caps_pad, int(out_off), int(col_off)


def build_pool_core(plan, c, ibatch, caps_pad, KMAX, total_cols):
    node_start = plan["node_start"]
    gcnt = np.bincount(ibatch, minlength=N_GRAPHS)
    g0 = c * GPC
    lo, hi = node_start[c], node_start[c + 1]
    kg = np.ceil(gcnt[g0:g0 + GPC] / POOL_SUB).astype(np.int64)
    local_ib = np.asarray(ibatch)[lo:hi]
    gs = np.searchsorted(local_ib, np.arange(g0, g0 + GPC + 1))
    slot_idx = []
    graph_order = []
    for kk in range(1, KMAX + 1):
        if caps_pad[kk] == 0:
            continue
        L = kk * POOL_SUB
        gsel = np.where(kg == kk)[0]
        for g in gsel:
            a, b = gs[g], gs[g + 1]
            n = b - a
            ids = np.empty(L, dtype=np.int64)
            ids[:n] = np.arange(a, b)
            ids[n:] = a
            slot_idx.append(ids)
            graph_order.append(g)
        for _ in range(int(caps_pad[kk] - len(gsel))):
            slot_idx.append(np.zeros(L, dtype=np.int64))
            graph_order.append(-1)
    flat = np.concatenate(slot_idx).astype(np.int16)
    assert len(flat) == total_cols * 16
    wrapped = np.empty((128, total_cols), dtype=np.int16)
    for p in range(128):
        wrapped[p, :] = flat[np.arange(total_cols) * 16 + (p % 16)]
    return wrapped, np.array(graph_order, dtype=np.int64)


def make_cfg_inputs(plan, inputs):
    ibatch = np.asarray(inputs["ibatch"])
    caps, KMAX = plan["pool"]["caps"], plan["pool"]["KMAX"]
    pool_ops, caps_pad, G_PAD, POOL_COLS = make_pool_ops(caps, KMAX)
    G_OUT = int(np.ceil(G_PAD / 128) * 128)
    KC_CELL = [128] * (DIM_CELL // 128) + ([DIM_CELL % 128] if DIM_CELL % 128 else [])

    cfg = dict(
        NT=plan["NT"], C=plan["C"], SH=plan["SH"], KCH=plan["KCH"],
        NGRP=plan["NGRP"], n_pad_nodes=plan["n_pad_nodes"],
        pool_ops=pool_ops, POOL_COLS=POOL_COLS,
        G_PAD=G_PAD, G_OUT=G_OUT, KC_CELL=KC_CELL,
    )

    x16 = np.asarray(inputs["drug_feature"], dtype=np.float16)
    gex = np.asarray(inputs["gexpr_data"], dtype=np.float32)
    wf32 = {k: np.ascontiguousarray(np.asarray(inputs[k], dtype=np.float32))
            for k in ("W2", "b1", "g1", "be1", "b2", "g2", "be2",
                      "bc1", "gc1", "bec1", "Wc2", "bc2")}
    W1_16 = np.ascontiguousarray(np.asarray(inputs["W1"], dtype=np.float16))
    Wc1_16 = np.ascontiguousarray(np.asarray(inputs["Wc1"], dtype=np.float16))

    in_maps = []
    orders = []
    NT, C, NGRP = plan["NT"], plan["C"], plan["NGRP"]
    spt = C * CHUNK
    for c in range(N_CORES):
        core = plan["cores"][c]
        pidx, gorder = build_pool_core(plan, c, ibatch, caps_pad, KMAX, POOL_COLS)
        orders.append(gorder)
        xs = np.zeros((plan["SH"], DIM_DRUG), dtype=np.float16)
        xs[:core["nreal"]] = x16[core["lo"]:core["hi"]]
        # pre-gathered slot-ordered x table, swizzled per gather group
        gathered = x16[core["src_slots"]]  # [NT*spt, 128]
        pad = NGRP * GMAX * spt - gathered.shape[0]
        if pad:
            gathered = np.concatenate(
                [gathered, np.zeros((pad, DIM_DRUG), np.float16)], 0)
        xsl = np.ascontiguousarray(
            gathered.reshape(NGRP, GMAX * C, CHUNK, DIM_DRUG)
            .transpose(0, 2, 1, 3).reshape(NGRP, CHUNK, GMAX * C * DIM_DRUG))
        gexT = np.ascontiguousarray(gex[c * GPC:(c + 1) * GPC].T.astype(np.float16))
        m = dict(
            x_slots=xsl, x_shard=xs, Sw=core["Sw"],
            idx_l2=core["idx_l2"], idx_loc=core["idx_loc"],
            s_flat=core["s_flat"].reshape(1, -1),
            pool_idx=pidx, gexprT=gexT, W1=W1_16, Wc1=Wc1_16,
            **wf32,
        )
        in_maps.append(m)

    def assemble(results):
        x_drug = np.full((N_GRAPHS, OUT), -np.inf, dtype=np.float32)
        x_cell = np.empty((N_GRAPHS, OUT), dtype=np.float32)
        for c in range(N_CORES):
            o = np.asarray(results[c]["out"])
            gorder = orders[c]
            rows = np.nonzero(gorder >= 0)[0]
            x_drug[c * GPC + gorder[rows]] = o[rows]
            x_cell[c * GPC:(c + 1) * GPC] = o[G_OUT:G_OUT + GPC]
        return x_drug, x_cell

    return cfg, in_maps, assemble


class _PartDone(Exception):
    pass


F32 = mybir.dt.float32
F16 = mybir.dt.float16
I32 = mybir.dt.int32
I16 = mybir.dt.int16
AF = mybir.ActivationFunctionType
ALU = mybir.AluOpType


def build_kernel(cfg):
    NT, C, SH, KCH, NGRP = cfg["NT"], cfg["C"], cfg["SH"], cfg["KCH"], cfg["NGRP"]
    TD, CH = TILE_DST, CHUNK
    NPAD = cfg["n_pad_nodes"]
    G_PAD, G_OUT = cfg["G_PAD"], cfg["G_OUT"]
    HH = OUT // 128
    NREAL = N_NODES
    groups = [(t0, min(t0 + GMAX, NT)) for t0 in range(0, NT, GMAX)]

    nc = bacc.Bacc(None, num_devices=N_CORES)

    # ---------------- parameters ----------------
    x_slots = nc.declare_dram_parameter("x_slots", [NGRP, CH, GMAX * C * DIM_DRUG], F16, isOutput=False)
    x_shard = nc.declare_dram_parameter("x_shard", [SH, DIM_DRUG], F16, isOutput=False)
    Swp = nc.declare_dram_parameter("Sw", [NGRP, CH, GMAX * KCH * TD], F16, isOutput=False)
    idx_l2 = nc.declare_dram_parameter("idx_l2", [CH, NT * C], I32, isOutput=False)
    idx_loc = nc.declare_dram_parameter("idx_loc", [CH, NT], I32, isOutput=False)
    s_flat = nc.declare_dram_parameter("s_flat", [1, SH], F16, isOutput=False)
    pool_idx = nc.declare_dram_parameter("pool_idx", [CH, cfg["POOL_COLS"]], I16, isOutput=False)
    gexprT = nc.declare_dram_parameter("gexprT", [DIM_CELL, GPC], F16, isOutput=False)
    W1p = nc.declare_dram_parameter("W1", [DIM_DRUG, HID], F16, isOutput=False)
    b1p = nc.declare_dram_parameter("b1", [HID], F32, isOutput=False)
    g1p = nc.declare_dram_parameter("g1", [HID], F32, isOutput=False)
    be1p = nc.declare_dram_parameter("be1", [HID], F32, isOutput=False)
    W2p_ = nc.declare_dram_parameter("W2", [HID, OUT], F32, isOutput=False)
    b2p = nc.declare_dram_parameter("b2", [OUT], F32, isOutput=False)
    g2p = nc.declare_dram_parameter("g2", [OUT], F32, isOutput=False)
    be2p = nc.declare_dram_parameter("be2", [OUT], F32, isOutput=False)
    Wc1p = nc.declare_dram_parameter("Wc1", [DIM_CELL, HID], F16, isOutput=False)
    bc1p = nc.declare_dram_parameter("bc1", [HID], F32, isOutput=False)
    gc1p = nc.declare_dram_parameter("gc1", [HID], F32, isOutput=False)
    bec1p = nc.declare_dram_parameter("bec1", [HID], F32, isOutput=False)
    Wc2p_ = nc.declare_dram_parameter("Wc2", [HID, OUT], F32, isOutput=False)
    bc2p_ = nc.declare_dram_parameter("bc2", [OUT], F32, isOutput=False)
    out = nc.declare_dram_parameter("out", [G_OUT + GPC, OUT], F32, isOutput=True)

    # ---------------- internal DRAM ----------------
    u1_shard = nc.dram_tensor("u1_shard", [SH, HID], F16)
    u1_full = nc.dram_tensor("u1_full", [N_CORES * SH, HID], F16, addr_space="Shared")
    u2pool = nc.dram_tensor("u2pool", [SH, OUT], F16)
    st1_in = nc.dram_tensor("st1_in", [128, 2], F32)
    st1_out = nc.dram_tensor("st1_out", [128, 2], F32, addr_space="Shared")
    st2_in = nc.dram_tensor("st2_in", [128, 6], F32)
    st2_out = nc.dram_tensor("st2_out", [128, 6], F32, addr_space="Shared")

    rg = [list(range(N_CORES))]

    with tile.TileContext(nc) as tc, ExitStack() as ctx:
      try:
        cpool = ctx.enter_context(tc.tile_pool(name="consts", bufs=1))
        rows_p = ctx.enter_context(tc.tile_pool(name="rows", bufs=3))
        sload_p = ctx.enter_context(tc.tile_pool(name="sload", bufs=2))
        sf_p = ctx.enter_context(tc.tile_pool(name="sf", bufs=2))
        work_p = ctx.enter_context(tc.tile_pool(name="work", bufs=4))
        tr_p = ctx.enter_context(tc.tile_pool(name="tr", bufs=2))
        stats_p = ctx.enter_context(tc.tile_pool(name="stats", bufs=1))
        cell_p = ctx.enter_context(tc.tile_pool(name="cell", bufs=2))
        pool_p = ctx.enter_context(tc.tile_pool(name="pool", bufs=2))
        psMM = ctx.enter_context(tc.tile_pool(name="psMM", bufs=5, space="PSUM"))
        psTR = ctx.enter_context(tc.tile_pool(name="psTR", bufs=2, space="PSUM"))
        psTRF = ctx.enter_context(tc.tile_pool(name="psTRF", bufs=1, space="PSUM"))

        # ---------------- constants ----------------
        identf = cpool.tile([128, 128], F32)
        make_identity(nc, identf[:])
        ident16 = cpool.tile([128, 128], F16)
        nc.vector.tensor_copy(ident16[:], identf[:])

        W1w = cpool.tile([128, HID], F16)
        nc.sync.dma_start(out=W1w[:], in_=W1p[:, :])
        W2 = cpool.tile([128, OUT], F32)
        nc.sync.dma_start(out=W2[:], in_=W2p_[:, :])
        W2f = cpool.tile([128, OUT], F16)
        Wc2 = cpool.tile([128, OUT], F32)
        nc.sync.dma_start(out=Wc2[:], in_=Wc2p_[:, :])
        Wc2f = cpool.tile([128, OUT], F16)

        idx2_sb = cpool.tile([128, NT * C], I32)
        nc.sync.dma_start(out=idx2_sb[:], in_=idx_l2[:, :])
        idxL_sb = cpool.tile([128, NT], I32)
        nc.sync.dma_start(out=idxL_sb[:], in_=idx_loc[:, :])
        rows0 = cpool.tile([128, NT * CH], F16)

        def col(param, n=128, off=0):
            t = cpool.tile([n, 1], F32, tag=f"col_{param.name}_{off}")
            nc.sync.dma_start(out=t[:], in_=param[off:off + n, None])
            return t

        b1c = col(b1p)
        g1c = col(g1p)
        be1c = col(be1p)
        b2c = [col(b2p, off=h * 128) for h in range(HH)]
        g2c = [col(g2p, off=h * 128) for h in range(HH)]
        be2c = [col(be2p, off=h * 128) for h in range(HH)]
        bc1c = col(bc1p)
        gc1c = col(gc1p)
        bec1c = col(bec1p)
        bc2c = [col(bc2p_, off=h * 128) for h in range(HH)]

        st1_sum = stats_p.tile([128, NT], F32)
        st1_sq = stats_p.tile([128, NT], F32)
        st2_sum = stats_p.tile([128, HH * NT], F32)
        st2_sq = stats_p.tile([128, HH * NT], F32)
        stc_sum = stats_p.tile([128, 2], F32)
        stc_sq = stats_p.tile([128, 2], F32)

        # ============ aggregation layer ============
        def agg_layer(idx_sb, table, self_table, post, tag, rows0_sb=None):
            """idx_sb None => rows streamed from pre-gathered x_slots (L1);
            else per-chunk [128,1]-offset indirect gathers from table (L2).
            rows0_sb: pre-gathered chunk-0 (local) rows; skip those gathers."""
            for gi, (t0, t1) in enumerate(groups):
                gl = t1 - t0
                rows = rows_p.tile([128, GMAX * C * CH], F16, tag=f"rows{tag}")
                if idx_sb is None:
                    nc.sync.dma_start(out=rows[:, :gl * C * CH],
                                      in_=x_slots[gi, :, :gl * C * CH])
                else:
                    for s in range(gl * C):
                        if rows0_sb is not None and s % C == 0:
                            continue
                        nc.gpsimd.indirect_dma_start(
                            out=rows[:, s * CH:(s + 1) * CH],
                            out_offset=None,
                            in_=table[:, :],
                            in_offset=bass.IndirectOffsetOnAxis(
                                ap=idx_sb[:, t0 * C + s:t0 * C + s + 1], axis=0),
                        )
                Sg = sload_p.tile([128, GMAX * KCH * TD], F16, tag=f"S{tag}")
                nc.sync.dma_start(out=Sg[:, :gl * KCH * TD],
                                  in_=Swp[gi, :, :gl * KCH * TD])
                for ti in range(gl):
                    t = t0 + ti
                    sf = sf_p.tile([128, NSELF, CH], F16, tag=f"sf{tag}")
                    nc.sync.dma_start(
                        out=sf[:],
                        in_=self_table[t * TD:(t + 1) * TD, :].rearrange(
                            "(h p) f -> p h f", p=CH))
                    zTf = psMM.tile([128, 512], F32, tag="mm", space="PSUM")
                    zT = zTf[:, :TD]
                    base = ti * KCH * TD
                    for h in range(NSELF):
                        nc.tensor.matmul(
                            zT, lhsT=sf[:, h, :],
                            rhs=Sg[:, base + h * TD:base + (h + 1) * TD],
                            start=(h == 0), stop=False)
                    for j in range(C):
                        if rows0_sb is not None and j == 0:
                            lhsT = rows0_sb[:, t * CH:(t + 1) * CH]
                        else:
                            lhsT = rows[:, (ti * C + j) * CH:(ti * C + j + 1) * CH]
                        nc.tensor.matmul(
                            zT, lhsT=lhsT,
                            rhs=Sg[:, base + (NSELF + j) * TD:base + (NSELF + j + 1) * TD],
                            start=False, stop=(j == C - 1))
                    post(t, zT)

        def l1_post(t, zT):
            zs = work_p.tile([128, TD], F16, tag="zs1")
            nc.scalar.copy(zs[:], zT)
            uTf = psMM.tile([128, 512], F32, tag="mm", space="PSUM")
            uT = uTf[:, :TD]
            nc.tensor.matmul(uT, lhsT=W1w[:], rhs=zs[:], start=True, stop=True)
            u1t = work_p.tile([128, TD], F16, tag="u1t")
            nc.scalar.activation(u1t[:], uT, AF.Relu, bias=b1c[:],
                                 accum_out=st1_sum[:, t:t + 1])
            sq = work_p.tile([128, TD], F16, tag="sq1")
            nc.scalar.activation(sq[:], u1t[:], AF.Square,
                                 accum_out=st1_sq[:, t:t + 1])
            rt2 = tr_p.tile([128, NSELF, CH], F16, tag="l1rt")
            for h in range(NSELF):
                pt = psTR.tile([128, CH], F16, tag="tr16", space="PSUM")
                nc.tensor.transpose(pt[:], u1t[:, h * CH:(h + 1) * CH], ident16[:])
                nc.vector.tensor_copy(rt2[:, h, :], pt[:])
            nc.sync.dma_start(
                out=u1_shard[t * TD:(t + 1) * TD, :].rearrange("(h p) f -> p h f", p=CH),
                in_=rt2[:])

        parts = cfg.get("parts", "all")
        agg_layer(None, None, x_shard, l1_post, "a")
        if parts == "l1":
            for q in range(17):
                tt = work_p.tile([128, 128], F16, tag="dbgcopy")
                nc.sync.dma_start(out=tt[:], in_=u1_shard[q * 128:(q + 1) * 128, :])
                cv = work_p.tile([128, 128], F32, tag="dbgcv")
                nc.vector.tensor_copy(cv[:], tt[:])
                nc.sync.dma_start(out=out[q * 128:(q + 1) * 128, :128], in_=cv[:])
            raise _PartDone()

        st1_red = stats_p.tile([128, 2], F32)
        nc.vector.tensor_reduce(st1_red[:, 0:1], st1_sum[:, :NT], mybir.AxisListType.X, ALU.add)
        nc.vector.tensor_reduce(st1_red[:, 1:2], st1_sq[:, :NT], mybir.AxisListType.X, ALU.add)
        nc.sync.dma_start(out=st1_in[:, :], in_=st1_red[:])

        # AR1 + AG (small first)
        tc.strict_bb_all_engine_barrier()
        nc.gpsimd.collective_compute(
            "AllReduce", ALU.add, replica_groups=rg,
            ins=[st1_in[:]], outs=[st1_out[:]])
        nc.gpsimd.collective_compute(
            "AllGather", ALU.bypass, replica_groups=rg,
            ins=[u1_shard[:]], outs=[u1_full[:]])
        # chunk-0 (local-src) L2 rows gathered from u1_shard — no AllGather
        # dependency, so these ~99 gathers hide inside the AG window
        for t in range(NT):
            nc.gpsimd.indirect_dma_start(
                out=rows0[:, t * CH:(t + 1) * CH],
                out_offset=None,
                in_=u1_shard[:, :],
                in_offset=bass.IndirectOffsetOnAxis(ap=idxL_sb[:, t:t + 1], axis=0),
            )
        st1_sb = stats_p.tile([128, 2], F32)
        nc.sync.dma_start(out=st1_sb[:], in_=st1_out[:, :])

        # ============ CELL pass 1 (tanh + stats) — fills the AG window ======
        KCs = cfg["KC_CELL"]
        n_bh = GPC // 512
        Wc1t = []
        koff = 0
        for ki, kk in enumerate(KCs):
            w = cpool.tile([128, HID], F16, tag=f"wc1_{ki}")
            nc.sync.dma_start(out=w[:kk, :], in_=Wc1p[koff:koff + kk, :])
            Wc1t.append(w)
            koff += kk
        cT = []
        for bh in range(n_bh):
            pc = psMM.tile([128, 512], F32, tag="mm", space="PSUM")
            koff = 0
            for ki, kk in enumerate(KCs):
                strip = cell_p.tile([128, 512], F16, tag="strip")
                nc.sync.dma_start(out=strip[:kk, :],
                                  in_=gexprT[koff:koff + kk, bh * 512:(bh + 1) * 512])
                nc.tensor.matmul(pc[:], lhsT=Wc1t[ki][:kk, :], rhs=strip[:kk, :],
                                 start=(ki == 0), stop=(ki == len(KCs) - 1))
                koff += kk
            ct = cell_p.tile([128, 512], F16, tag=f"cellct{bh}")
            nc.scalar.activation(ct[:], pc[:], AF.Tanh, bias=bc1c[:],
                                 accum_out=stc_sum[:, bh:bh + 1])
            csq = cell_p.tile([128, 512], F16, tag="cellsq")
            nc.scalar.activation(csq[:], ct[:], AF.Square,
                                 accum_out=stc_sq[:, bh:bh + 1])
            cT.append(ct)

        def bn_affine(sum_c, sq_c, gc, bec, bias_relu_col, n_real, n_pad, pfx):
            a_c = cpool.tile([128, 1], F32, tag=f"{pfx}_a")
            c_c = cpool.tile([128, 1], F32, tag=f"{pfx}_c")
            m_c = cpool.tile([128, 1], F32, tag=f"{pfx}_m")
            q_c = cpool.tile([128, 1], F32, tag=f"{pfx}_q")
            t1 = cpool.tile([128, 1], F32, tag=f"{pfx}_t1")
            if bias_relu_col is not None:
                rb = cpool.tile([128, 1], F32, tag=f"{pfx}_rb")
                nc.scalar.activation(rb[:], bias_relu_col[:], AF.Relu)
                rb2 = cpool.tile([128, 1], F32, tag=f"{pfx}_rb2")
                nc.scalar.activation(rb2[:], rb[:], AF.Square)
                nc.vector.tensor_scalar(out=m_c[:], in0=rb[:], scalar1=float(-n_pad),
                                        scalar2=None, op0=ALU.mult)
                nc.vector.tensor_tensor(out=m_c[:], in0=m_c[:], in1=sum_c, op=ALU.add)
                nc.vector.tensor_scalar(out=m_c[:], in0=m_c[:], scalar1=1.0 / n_real,
                                        scalar2=None, op0=ALU.mult)
                nc.vector.tensor_scalar(out=q_c[:], in0=rb2[:], scalar1=float(-n_pad),
                                        scalar2=None, op0=ALU.mult)
                nc.vector.tensor_tensor(out=q_c[:], in0=q_c[:], in1=sq_c, op=ALU.add)
                nc.vector.tensor_scalar(out=q_c[:], in0=q_c[:], scalar1=1.0 / n_real,
                                        scalar2=None, op0=ALU.mult)
            else:
                nc.vector.tensor_scalar(out=m_c[:], in0=sum_c, scalar1=1.0 / n_real,
                                        scalar2=None, op0=ALU.mult)
                nc.vector.tensor_scalar(out=q_c[:], in0=sq_c, scalar1=1.0 / n_real,
                                        scalar2=None, op0=ALU.mult)
            nc.scalar.activation(t1[:], m_c[:], AF.Square)
            nc.vector.tensor_tensor(out=t1[:], in0=q_c[:], in1=t1[:], op=ALU.subtract)
            nc.vector.tensor_scalar(out=t1[:], in0=t1[:], scalar1=float(EPS),
                                    scalar2=None, op0=ALU.add)
            nc.vector.reciprocal(t1[:], t1[:])
            nc.scalar.activation(t1[:], t1[:], AF.Sqrt)
            nc.vector.tensor_tensor(out=a_c[:], in0=gc[:], in1=t1[:], op=ALU.mult)
            nc.vector.tensor_tensor(out=c_c[:], in0=m_c[:], in1=a_c[:], op=ALU.mult)
            nc.vector.tensor_tensor(out=c_c[:], in0=bec[:], in1=c_c[:], op=ALU.subtract)
            return a_c, c_c

        a1c, c1c = bn_affine(st1_sb[:, 0:1], st1_sb[:, 1:2], g1c, be1c, b1c,
                             NREAL, NPAD, "bn1")

        # W2' = a1 * W2 (fp16); r2 rows via fp32 matmul then cast
        nc.vector.tensor_scalar(out=W2f[:], in0=W2[:], scalar1=a1c[:],
                                scalar2=None, op0=ALU.mult)
        rr16 = []
        for h in range(HH):
            pr = psTRF.tile([128, 128], F32, tag="trf", space="PSUM")
            nc.tensor.matmul(pr[:1, :], lhsT=c1c[:], rhs=W2[:, h * 128:(h + 1) * 128],
                             start=True, stop=True)
            rr = cpool.tile([1, 128], F16, tag=f"r2_{h}")
            nc.vector.tensor_copy(rr[:], pr[:1, :])
            rr16.append(rr)

        if parts == "l1c":
            raise _PartDone()

        # ============ LAYER 2 ============
        def l2_post(t, zT):
            zs = work_p.tile([128, TD], F16, tag="zs2")
            nc.scalar.copy(zs[:], zT)
            s_t = sf_p.tile([1, TD], F16, tag="s_t")
            nc.sync.dma_start(out=s_t[:], in_=s_flat[0:1, t * TD:(t + 1) * TD])
            rt4 = tr_p.tile([128, NSELF, HH, CH], F16, tag="l2rt")
            for h in range(HH):
                uTf = psMM.tile([128, 512], F32, tag="mm", space="PSUM")
                uT = uTf[:, :TD]
                nc.tensor.matmul(uT, lhsT=W2f[:, h * 128:(h + 1) * 128],
                                 rhs=zs[:], start=True, stop=False)
                nc.tensor.matmul(uT, lhsT=rr16[h][:],
                                 rhs=s_t[:], start=False, stop=True)
                u2t = work_p.tile([128, TD], F16, tag="u2t")
                nc.scalar.activation(u2t[:], uT, AF.Relu, bias=b2c[h][:],
                                     accum_out=st2_sum[:, t * HH + h:t * HH + h + 1])
                sq = work_p.tile([128, TD], F16, tag="sq2")
                nc.scalar.activation(sq[:], u2t[:], AF.Square,
                                     accum_out=st2_sq[:, t * HH + h:t * HH + h + 1])
                for q in range(TD // CH):
                    pt = psTR.tile([128, CH], F16, tag="tr16", space="PSUM")
                    nc.tensor.transpose(pt[:], u2t[:, q * CH:(q + 1) * CH], ident16[:])
                    nc.vector.tensor_copy(rt4[:, q, h, :], pt[:])
            nc.sync.dma_start(
                out=u2pool[t * TD:(t + 1) * TD, :].rearrange(
                    "(q p) (h f) -> p q h f", p=CH, h=HH),
                in_=rt4[:])

        agg_layer(idx2_sb, u1_full, u1_shard, l2_post, "b", rows0_sb=rows0)

        if parts == "nol2stats":
            raise _PartDone()
        st2_red = stats_p.tile([128, 6], F32)
        for h in range(HH):
            nc.vector.tensor_reduce(
                st2_red[:, 2 * h:2 * h + 1],
                st2_sum[:].rearrange("p (t h) -> p t h", h=HH)[:, :, h],
                mybir.AxisListType.X, ALU.add)
            nc.vector.tensor_reduce(
                st2_red[:, 2 * h + 1:2 * h + 2],
                st2_sq[:].rearrange("p (t h) -> p t h", h=HH)[:, :, h],
                mybir.AxisListType.X, ALU.add)
        nc.vector.tensor_reduce(st2_red[:, 4:5], stc_sum[:, :], mybir.AxisListType.X, ALU.add)
        nc.vector.tensor_reduce(st2_red[:, 5:6], stc_sq[:, :], mybir.AxisListType.X, ALU.add)
        nc.sync.dma_start(out=st2_in[:, :], in_=st2_red[:])
        tc.strict_bb_all_engine_barrier()
        nc.gpsimd.collective_compute(
            "AllReduce", ALU.add, replica_groups=rg,
            ins=[st2_in[:]], outs=[st2_out[:]])
        st2_sb = stats_p.tile([128, 6], F32)
        nc.sync.dma_start(out=st2_sb[:], in_=st2_out[:, :])

        a2c, c2c = [], []
        for h in range(HH):
            a_, c_ = bn_affine(st2_sb[:, 2 * h:2 * h + 1], st2_sb[:, 2 * h + 1:2 * h + 2],
                               g2c[h], be2c[h], b2c[h], NREAL, NPAD, f"bn2_{h}")
            a2c.append(a_)
            c2c.append(c_)

        # ---- cell finish (BNc affine + second linear), overlaps pool ----
        acc_, ccc_ = bn_affine(st2_sb[:, 4:5], st2_sb[:, 5:6], gc1c, bec1c, None,
                               N_GRAPHS, 0, "bnc")
        nc.vector.tensor_scalar(out=Wc2f[:], in0=Wc2[:], scalar1=acc_[:],
                                scalar2=None, op0=ALU.mult)
        bc2f = []
        for h in range(HH):
            pb = psTRF.tile([128, 128], F32, tag="trf", space="PSUM")
            nc.tensor.matmul(pb[:, :1], lhsT=Wc2[:, h * 128:(h + 1) * 128], rhs=ccc_[:],
                             start=True, stop=True)
            bb_ = cpool.tile([128, 1], F32, tag=f"bc2f_{h}")
            nc.vector.tensor_tensor(out=bb_[:], in0=pb[:, :1], in1=bc2c[h][:], op=ALU.add)
            bc2f.append(bb_)
        for bh in range(n_bh):
            for h in range(HH):
                px = psMM.tile([128, 512], F32, tag="mm", space="PSUM")
                nc.tensor.matmul(px[:], lhsT=Wc2f[:, h * 128:(h + 1) * 128],
                                 rhs=cT[bh][:], start=True, stop=True)
                xc = cell_p.tile([128, 512], F32, tag="cellxc")
                nc.scalar.activation(xc[:], px[:], AF.Relu, bias=bc2f[h][:])
                rtc = tr_p.tile([128, 4, 128], F32, tag="cellrt")
                for bb in range(4):
                    ptf = psTRF.tile([128, 128], F32, tag="trf", space="PSUM")
                    nc.tensor.transpose(ptf[:], xc[:, bb * 128:(bb + 1) * 128], identf[:])
                    nc.vector.tensor_copy(rtc[:, bb, :], ptf[:])
                nc.scalar.dma_start(
                    out=out[G_OUT + bh * 512:G_OUT + (bh + 1) * 512,
                            h * 128:(h + 1) * 128].rearrange("(b p) f -> p b f", p=CH),
                    in_=rtc[:])

        if parts == "nopool":
            raise _PartDone()
        # ============ POOL ============
        pooled = stats_p.tile([128, HH, G_PAD], F32)
        for (n_idx, n_g, L, col_off, out_off) in cfg["pool_ops"]:
            pidx = pool_p.tile([128, n_idx // 16], I16, tag="pidx")
            nc.sync.dma_start(out=pidx[:], in_=pool_idx[:, col_off:col_off + n_idx // 16])
            gath = pool_p.tile([128, HH, n_idx], F16, tag="pgath")
            nc.gpsimd.dma_gather(
                gath[:], u2pool[:, :], pidx[:], n_idx, n_idx,
                elem_size=OUT, transpose=True, single_packet=False)
            nc.vector.tensor_reduce(
                pooled[:, :, out_off:out_off + n_g],
                gath[:].rearrange("p h (g l) -> p h g l", g=n_g),
                mybir.AxisListType.X, ALU.max)
        for h in range(HH):
            nc.vector.tensor_scalar(
                out=pooled[:, h, :], in0=pooled[:, h, :],
                scalar1=a2c[h][:], scalar2=c2c[h][:], op0=ALU.mult, op1=ALU.add)
        for h in range(HH):
            for q in range(G_OUT // 128):
                n_here = min(128, G_PAD - q * 128)
                if n_here <= 0:
                    break
                ptf = psTRF.tile([128, 128], F32, tag="trf", space="PSUM")
                nc.tensor.transpose(ptf[:n_here, :], pooled[:, h, q * 128:q * 128 + n_here],
                                    identf[:])
                rt = tr_p.tile([128, 128], F32, tag="poolrt")
                nc.vector.tensor_copy(rt[:n_here, :], ptf[:n_here, :])
                nc.sync.dma_start(
                    out=out[q * 128:q * 128 + n_here, h * 128:(h + 1) * 128],
                    in_=rt[:n_here, :])

      except _PartDone:
          pass
    nc.compile()
    return nc


_BUILD_CACHE = {}


def _run(inputs, trace=False, parts="all"):
    plan = build_plan(inputs["drug_adj"], inputs["ibatch"])
    cfg, in_maps, assemble = make_cfg_inputs(plan, inputs)
    cfg["parts"] = parts
    key = (cfg["NT"], cfg["C"], cfg["G_PAD"], parts, tuple(map(tuple, cfg["pool_ops"])))
    if key not in _BUILD_CACHE:
        _BUILD_CACHE[key] = build_kernel(cfg)
    nc = _BUILD_CACHE[key]
    res = run_bass_kernel_spmd(nc, in_maps, core_ids=list(range(8)), trace=trace)
    x_drug, x_cell = assemble(res.results)
    return (x_drug, x_cell), res


def kernel(**inputs):
    inputs = {k: np.asarray(v) for k, v in inputs.items()}
    (x_drug, x_cell), _ = _run(inputs, trace=False)
    return x_drug, x_cell
